# revision 33
# baseline (speedup 1.0000x reference)
# Trainium2 Bass kernel for CustomFullyConnectedLayer:
#   y = x @ W.T,  W[(c+i)%N, c] += V[i, c] for i in diag_pos  (banded weight)
# Strategy: data-parallel over batch across 8 cores; compute y.T directly
# from host-pre-transposed x so no on-chip transposes are needed.
#   y.T[r, b] = sum_i V[i, r-i] * x.T[(r-i)%N, b]
# Per 128-row output tile j (r = 128j+q):
#   psum[q, b] = sum_p A_j[p, q] xT[128j+p, b] + sum_pp B_j[pp, q] xT[128(j-1)+64+pp, b]
# A_j: in-tile band (i <= q); B_j: 64-row wrap band from the previous
# feature tile (i > q), zero-padded to base partition 64. Both host-built.
# The PE HAM clock gate is cold (1.2 GHz) by default and warms to 2.4 GHz
# after ~3.4us of sustained activity; the kernel keeps the matmul stream
# gapless (deep psum pool, big copies, batched stores) so it stays warm.
import os
import sys

import numpy as np

if "/opt/trn_rl_repo" not in sys.path:
    sys.path.insert(0, "/opt/trn_rl_repo")

import ml_dtypes

BATCH = 8192
N = 3072
NCORES = 8
BC = BATCH // NCORES          # 1024 batch cols per core
NJ = N // 128                 # 24 feature/output tiles
HB = BC // 2                  # psum half: one fp32 bank
SG = 3                        # output tiles per store group

_CACHE = {}
LAST_RESULTS = None


def _build_program():
    import concourse.mybir as mybir
    import concourse.tile as tile
    from concourse import bacc

    cdt = mybir.dt.bfloat16
    f32 = mybir.dt.float32

    nc = bacc.Bacc("TRN2", target_bir_lowering=False, debug=False)
    xT = nc.dram_tensor("xT", [128, NJ, BC], cdt, kind="ExternalInput")
    # wext[:, j, 0:128] = in-tile band A_j; wext[:, j, 128:160] = spill of
    # c-tile j onto the first 32 rows of output tile j+1 (wrap band).
    wext = nc.dram_tensor("wext", [128, NJ, 160], cdt, kind="ExternalInput")
    yT = nc.dram_tensor("yT", [128, NJ, BC], cdt, kind="ExternalOutput")

    with tile.TileContext(nc) as tc:
        with (
            tc.tile_pool(name="consts", bufs=1) as consts,
            tc.tile_pool(name="yout", bufs=3) as yout,
            tc.tile_pool(name="ps", bufs=4, space="PSUM") as psp,
        ):
            w_sb = consts.tile([128, NJ, 160], cdt)
            x_sb = consts.tile([128, NJ, BC], cdt)
            wsrc = consts.tile([128, 128], cdt)

            # One load queue: concurrent queues fair-share the DMA engines
            # per packet, so ordering is only guaranteed within a queue.
            # Weights interleave ahead of the x tiles they gate. Every
            # matmul is full K=128: K<128 never lifts the HAM clock gate.
            def wload(g):
                nc.sync.dma_start(
                    out=w_sb[:, 6 * g:6 * (g + 1), :],
                    in_=wext[:, 6 * g:6 * (g + 1), :],
                )

            def xload(j0, nj):
                nc.sync.dma_start(
                    out=x_sb[:, j0:j0 + nj, :], in_=xT[:, j0:j0 + nj, :]
                )

            wload(0)
            xload(NJ - 2, 2)   # j=0 needs tile 23 for the wrap band
            xload(0, 2)
            wload(1)
            xload(2, 2)
            wload(2)
            xload(4, 2)
            wload(3)
            xload(6, 2)
            xload(8, 4)
            xload(12, 4)
            xload(16, 4)
            xload(20, 2)

            # PE warm-up on zeros: ~3.4us of sustained activity lifts the
            # HAM clock gate from 1.2 to 2.4 GHz while the DMAs land.
            nc.vector.memset(wsrc, 0.0)
            wps = psp.tile([128, BC], f32, tag="ps")
            for _ in range(40):
                nc.tensor.matmul(
                    wps[:, :128], lhsT=wsrc, rhs=wsrc, start=True, stop=True
                )

            for j in range(NJ):
                jm1 = (j - 1) % NJ
                ps = psp.tile([128, BC], f32, tag="ps")
                nc.tensor.matmul(
                    ps[:, 0:HB], lhsT=w_sb[:, j, 0:128],
                    rhs=x_sb[:, j, 0:HB],
                    start=True, stop=True, skip_group_check=True,
                )
                nc.tensor.matmul(
                    ps[:, HB:BC], lhsT=w_sb[:, j, 0:128],
                    rhs=x_sb[:, j, HB:BC],
                    start=True, stop=True, skip_group_check=True,
                )
                nc.tensor.matmul(
                    ps[0:32, 0:HB], lhsT=w_sb[:, jm1, 128:160],
                    rhs=x_sb[:, jm1, 0:HB],
                    start=False, stop=True, skip_group_check=True,
                )
                nc.tensor.matmul(
                    ps[0:32, HB:BC], lhsT=w_sb[:, jm1, 128:160],
                    rhs=x_sb[:, jm1, HB:BC],
                    start=False, stop=True, skip_group_check=True,
                )
                u = j % SG
                if u == 0:
                    y_gb = yout.tile([128, SG, BC], cdt)
                # one whole-j copy, alternating engines
                if j % 2 == 0:
                    nc.scalar.copy(out=y_gb[:, u, :], in_=ps)
                else:
                    nc.vector.tensor_copy(out=y_gb[:, u, :], in_=ps)
                if u == SG - 1:
                    g = j // SG
                    if g < NJ // SG - 1:
                        nc.gpsimd.dma_start(
                            out=yT[:, SG * g:SG * (g + 1), :], in_=y_gb
                        )
                    else:
                        # split the last group so the drain overlaps
                        for v in range(SG):
                            nc.gpsimd.dma_start(
                                out=yT[:, SG * g + v, :], in_=y_gb[:, v, :]
                            )

    nc.compile()
    return nc


def _host_prep(x, V, diag_pos):
    bf16 = ml_dtypes.bfloat16
    diag = np.asarray(diag_pos).astype(np.int64) % N
    if diag.size and int(diag.max()) > 29:
        raise ValueError(
            f"band kernel supports diag offsets <= 29, got {int(diag.max())}"
        )
    V32 = np.asarray(V, dtype=np.float32)

    # wext[j][p, 0:128] = A_j (in-tile band), wext[j][p, 128+q] = spill of
    # c-tile j onto rows q<29 of output tile j+1.
    W = np.zeros((NJ, 128, 160), np.float32)
    jj = np.arange(NJ)[:, None]
    for i in diag:
        i = int(i)
        p = np.arange(128 - i)[None, :]
        W[jj, p, p + i] += V32[i, (128 * jj + p) % N]
        if i > 0:
            pw = np.arange(128 - i, 128)[None, :]
            W[jj, pw, pw + i] += V32[i, (128 * jj + pw) % N]
    wext = np.ascontiguousarray(W.transpose(1, 0, 2)).astype(bf16)

    xb = np.ascontiguousarray(np.asarray(x, dtype=np.float32)).astype(bf16)
    xb = xb.view(np.uint16)
    # partition-major per core: xT[p, j, b] = x.T[128j+p, b]
    xTs = [
        np.ascontiguousarray(
            xb[k * BC:(k + 1) * BC, :].reshape(BC, NJ, 128).transpose(2, 1, 0)
        ).view(bf16)
        for k in range(NCORES)
    ]
    return xTs, wext


def kernel(x, V, diag_pos):
    global LAST_RESULTS
    from concourse.bass_utils import run_bass_kernel_spmd

    if "prog" not in _CACHE:
        _CACHE["prog"] = _build_program()
    nc = _CACHE["prog"]

    xTs, wext = _host_prep(x, V, diag_pos)
    in_maps = [
        {"xT": xTs[k], "wext": wext} for k in range(NCORES)
    ]
    res = run_bass_kernel_spmd(nc, in_maps, core_ids=list(range(NCORES)))
    LAST_RESULTS = res
    out = np.empty((BATCH, N), np.float32)
    for k in range(NCORES):
        # yT[q, j, b] = y.T[128j+q, b]  ->  y[b, 128j+q]
        arr = np.asarray(res.results[k]["yT"]).astype(np.float32)
        out[k * BC:(k + 1) * BC, :] = arr.transpose(2, 1, 0).reshape(BC, N)
    return out


# revision 36
# speedup vs baseline: 1.1578x; 1.1578x over previous
# Trainium2 Bass kernel for CustomFullyConnectedLayer:
#   y = x @ W.T,  W[(c+i)%N, c] += V[i, c] for i in diag_pos  (banded weight)
# Strategy: data-parallel over batch across 8 cores; compute y.T directly
# from host-pre-transposed x so no on-chip transposes are needed.
#   y.T[r, b] = sum_i V[i, r-i] * x.T[(r-i)%N, b]
# Per 128-row output tile j (r = 128j+q):
#   psum[q, b] = sum_p A_j[p, q] xT[128j+p, b] + sum_pp B_j[pp, q] xT[128(j-1)+64+pp, b]
# A_j: in-tile band (i <= q); B_j: 64-row wrap band from the previous
# feature tile (i > q), zero-padded to base partition 64. Both host-built.
# The PE HAM clock gate is cold (1.2 GHz) by default and warms to 2.4 GHz
# after ~3.4us of sustained activity; the kernel keeps the matmul stream
# gapless (deep psum pool, big copies, batched stores) so it stays warm.
import os
import sys

import numpy as np

if "/opt/trn_rl_repo" not in sys.path:
    sys.path.insert(0, "/opt/trn_rl_repo")

import ml_dtypes

BATCH = 8192
N = 3072
NCORES = 8
BC = BATCH // NCORES          # 1024 batch cols per core
NJ = N // 128                 # 24 feature/output tiles
HB = BC // 2                  # psum half: one fp32 bank
SG = 2                        # output tiles per store group

_CACHE = {}
LAST_RESULTS = None


def _build_program():
    import concourse.mybir as mybir
    import concourse.tile as tile
    from concourse import bacc

    cdt = mybir.dt.bfloat16
    f32 = mybir.dt.float32

    nc = bacc.Bacc("TRN2", target_bir_lowering=False, debug=False)
    xT = nc.dram_tensor("xT", [128, NJ, BC], cdt, kind="ExternalInput")
    # wext[:, j, 0:128] = in-tile band A_j; wext[:, j, 128:160] = spill of
    # c-tile j onto the first 32 rows of output tile j+1 (wrap band).
    wext = nc.dram_tensor("wext", [128, NJ, 160], cdt, kind="ExternalInput")
    yT = nc.dram_tensor("yT", [128, NJ, BC], cdt, kind="ExternalOutput")

    with tile.TileContext(nc) as tc:
        with (
            tc.tile_pool(name="consts", bufs=1) as consts,
            tc.tile_pool(name="yout", bufs=3) as yout,
            tc.tile_pool(name="ps", bufs=4, space="PSUM") as psp,
        ):
            w_sb = consts.tile([128, NJ, 160], cdt)
            x_sb = consts.tile([128, NJ, BC], cdt)
            wsrc = consts.tile([128, 128], cdt)

            # Everything on ONE queue: the DMA engines drain queues in
            # transfer-granularity FIFO order, so a single queue is the only
            # way to control the global arrival order (loads ahead of the
            # matmuls they gate, stores trickling in between instead of
            # stealing half the bandwidth). Every matmul is full K=128:
            # K<128 never lifts the HAM clock gate.
            def wload(g):
                nc.sync.dma_start(
                    out=w_sb[:, 6 * g:6 * (g + 1), :],
                    in_=wext[:, 6 * g:6 * (g + 1), :],
                )

            def xload(j0, nj):
                nc.sync.dma_start(
                    out=x_sb[:, j0:j0 + nj, :], in_=xT[:, j0:j0 + nj, :]
                )

            wload(0)
            xload(NJ - 2, 2)   # j=0 needs tile 23 for the wrap band
            xload(0, 2)
            wload(1)
            xload(2, 2)
            wload(2)
            xload(4, 2)
            wload(3)
            xload(6, 2)
            xload(8, 2)

            # PE warm-up on zeros: ~3.4us of sustained activity lifts the
            # HAM clock gate from 1.2 to 2.4 GHz while the DMAs land.
            nc.vector.memset(wsrc, 0.0)
            wps = psp.tile([128, BC], f32, tag="ps")
            for _ in range(40):
                nc.tensor.matmul(
                    wps[:, :128], lhsT=wsrc, rhs=wsrc, start=True, stop=True
                )

            for j in range(NJ):
                jm1 = (j - 1) % NJ
                ps = psp.tile([128, BC], f32, tag="ps")
                nc.tensor.matmul(
                    ps[:, 0:HB], lhsT=w_sb[:, j, 0:128],
                    rhs=x_sb[:, j, 0:HB],
                    start=True, stop=True, skip_group_check=True,
                )
                nc.tensor.matmul(
                    ps[:, HB:BC], lhsT=w_sb[:, j, 0:128],
                    rhs=x_sb[:, j, HB:BC],
                    start=True, stop=True, skip_group_check=True,
                )
                nc.tensor.matmul(
                    ps[0:32, 0:HB], lhsT=w_sb[:, jm1, 128:160],
                    rhs=x_sb[:, jm1, 0:HB],
                    start=False, stop=True, skip_group_check=True,
                )
                nc.tensor.matmul(
                    ps[0:32, HB:BC], lhsT=w_sb[:, jm1, 128:160],
                    rhs=x_sb[:, jm1, HB:BC],
                    start=False, stop=True, skip_group_check=True,
                )
                u = j % SG
                if u == 0:
                    y_gb = yout.tile([128, SG, BC], cdt)
                # one whole-j copy, alternating engines
                if j % 2 == 0:
                    nc.scalar.copy(out=y_gb[:, u, :], in_=ps)
                else:
                    nc.vector.tensor_copy(out=y_gb[:, u, :], in_=ps)
                if u == SG - 1:
                    g = j // SG
                    if g == NJ // SG - 1:
                        # split the last group so the drain overlaps
                        for v in range(SG):
                            nc.sync.dma_start(
                                out=yT[:, SG * g + v, :], in_=y_gb[:, v, :]
                            )
                    else:
                        nc.sync.dma_start(
                            out=yT[:, SG * g:SG * (g + 1), :], in_=y_gb
                        )
                    # next x pair goes on the queue right after this store
                    nxt = 10 + 2 * g
                    if nxt < NJ - 2:
                        xload(nxt, 2)

    nc.compile()
    return nc


def _host_prep(x, V, diag_pos):
    bf16 = ml_dtypes.bfloat16
    diag = np.asarray(diag_pos).astype(np.int64) % N
    if diag.size and int(diag.max()) > 29:
        raise ValueError(
            f"band kernel supports diag offsets <= 29, got {int(diag.max())}"
        )
    V32 = np.asarray(V, dtype=np.float32)

    # wext[j][p, 0:128] = A_j (in-tile band), wext[j][p, 128+q] = spill of
    # c-tile j onto rows q<29 of output tile j+1.
    W = np.zeros((NJ, 128, 160), np.float32)
    jj = np.arange(NJ)[:, None]
    for i in diag:
        i = int(i)
        p = np.arange(128 - i)[None, :]
        W[jj, p, p + i] += V32[i, (128 * jj + p) % N]
        if i > 0:
            pw = np.arange(128 - i, 128)[None, :]
            W[jj, pw, pw + i] += V32[i, (128 * jj + pw) % N]
    wext = np.ascontiguousarray(W.transpose(1, 0, 2)).astype(bf16)

    xb = np.ascontiguousarray(np.asarray(x, dtype=np.float32)).astype(bf16)
    xb = xb.view(np.uint16)
    # partition-major per core: xT[p, j, b] = x.T[128j+p, b]
    xTs = [
        np.ascontiguousarray(
            xb[k * BC:(k + 1) * BC, :].reshape(BC, NJ, 128).transpose(2, 1, 0)
        ).view(bf16)
        for k in range(NCORES)
    ]
    return xTs, wext


def kernel(x, V, diag_pos):
    global LAST_RESULTS
    from concourse.bass_utils import run_bass_kernel_spmd

    if "prog" not in _CACHE:
        _CACHE["prog"] = _build_program()
    nc = _CACHE["prog"]

    xTs, wext = _host_prep(x, V, diag_pos)
    in_maps = [
        {"xT": xTs[k], "wext": wext} for k in range(NCORES)
    ]
    res = run_bass_kernel_spmd(nc, in_maps, core_ids=list(range(NCORES)))
    LAST_RESULTS = res
    out = np.empty((BATCH, N), np.float32)
    for k in range(NCORES):
        # yT[q, j, b] = y.T[128j+q, b]  ->  y[b, 128j+q]
        arr = np.asarray(res.results[k]["yT"]).astype(np.float32)
        out[k * BC:(k + 1) * BC, :] = arr.transpose(2, 1, 0).reshape(BC, N)
    return out
